# revision 1
# baseline (speedup 1.0000x reference)
"""Causal self-attention Bass kernel for 8 trn2 NeuronCores.

Problem: B=4, T=2048, D=1024, H=16 causal self-attention (qkv proj + attn + out proj).

Sharding: core c = 2*b + g handles batch b (=c//2) and head-group g (=c%2, 8 heads).
Per core:
  - qkv projection column-shard: q,k,v columns for its 8 heads only.
  - flash-style attention in transposed-score layout sT[tk, tq]; softmax denominator
    via an extra ones-column in the AV matmul (row 64 of the [65, 512] psum output).
  - output projection row-shard (w_proj rows for its head dims) -> partial [T, D].
  - pairwise ReduceScatter {2b, 2b+1} sums the two head-group partials and splits
    output rows t: even core -> rows [0,1024), odd -> [1024, 2048).
Host reassembles by stacking the two halves per batch.

Precision: matmuls run as float32r (1 cyc/row for N>=256). Q/K path additionally
uses bf16 storage for xT / w_qk (softmax is shift-robust: score errors are absolute
and scores are O(1)). Value path (v, attn weights, projections) stays f32/f32r.
b_v is folded into beta = b_proj(once per pair) + w_proj_shard.T @ b_v_shard since
softmax rows sum to 1.
"""

from contextlib import ExitStack

import ml_dtypes
import numpy as np

import concourse.bass as bass
import concourse.mybir as mybir
import concourse.tile as tile
from concourse import bacc
from concourse.bass_utils import run_bass_kernel_spmd

B, T, D, H = 4, 2048, 1024, 16
HD = D // H  # 64
NCORES = 8
P = 128
f32 = mybir.dt.float32
f32r = mybir.dt.float32r
bf16 = mybir.dt.bfloat16
EXP = mybir.ActivationFunctionType.Exp
LN = mybir.ActivationFunctionType.Ln

_CACHE = {}
LAST_RESULTS = None
_DEBUG_SINK = None


def _dbg(nc, name, ap):
    if _DEBUG_SINK is not None and name in _DEBUG_SINK:
        nc.sync.dma_start(_DEBUG_SINK[name].ap(), ap)


def _emit(nc, tc, x_d, wqk_d, wv_d, bqk_d, wproj_d, beta_d, out_d):
    with ExitStack() as ctx:
        # ---------------- constants / persistent tiles ----------------
        const = ctx.enter_context(tc.tile_pool(name="const", bufs=1))
        bootc = ctx.enter_context(tc.tile_pool(name="boot", bufs=1))
        ident_f = bootc.tile([P, P], bf16, tag="ident_f")
        nc.gpsimd.memset(ident_f[:], 0.0)
        nc.gpsimd.affine_select(
            out=ident_f[:], in_=ident_f[:],
            compare_op=mybir.AluOpType.not_equal, fill=1.0,
            base=0, pattern=[[-1, P]], channel_multiplier=1,
        )
        ident = const.tile([P, P], f32r, tag="ident")
        nc.vector.tensor_copy(ident[:], ident_f[:])
        # (boot tiles stay resident; ~2.5KB)
        # triangle mask [128,128]: keep (1.0) iff f >= p
        mask_tri = const.tile([P, P], bf16, tag="mask_tri")
        nc.gpsimd.memset(mask_tri[:], 1.0)
        nc.gpsimd.affine_select(
            out=mask_tri[:], in_=mask_tri[:],
            compare_op=mybir.AluOpType.is_ge, fill=0.0,
            base=0, pattern=[[1, P]], channel_multiplier=-1,
        )
        bq = [const.tile([P, 1], f32, tag=f"bq{m}", name=f"bq{m}") for m in range(8)]
        beta_b = const.tile([P, D], bf16, tag="beta_b")

        def _load_small_consts():
            for m in range(8):
                nc.sync.dma_start(bq[m][:], bqk_d.ap()[m])
            nc.sync.dma_start(beta_b[0:1, :], beta_d.ap())
            nc.gpsimd.partition_broadcast(beta_b[:], beta_b[0:1, :], channels=P)
        # w_proj pool reserved here; its DMAs are emitted after phase 1 starts
        # so the x loads win the DMA queue.
        wpp = ctx.enter_context(tc.tile_pool(name="wpp", bufs=1))
        wproj_t = [wpp.tile([P, D], f32r, tag=f"wp{hp}", name=f"wp{hp}") for hp in range(4)]
        _dbg(nc, "beta_b", beta_b[:])

        # persistent activations
        xt_pool = ctx.enter_context(tc.tile_pool(name="xt", bufs=1))
        xT = [xt_pool.tile([P, T], bf16, tag=f"xT{k}", name=f"xT{k}") for k in range(8)]
        vv_pool = ctx.enter_context(tc.tile_pool(name="vv", bufs=1))
        vv = [vv_pool.tile([P, 520], f32r, tag=f"vv{i}", name=f"vv{i}") for i in range(16)]
        on_pool = ctx.enter_context(tc.tile_pool(name="outn", bufs=1))
        outN = [[on_pool.tile([P, 512], f32r, tag=f"outN{mp}J{J}", name=f"outN{mp}J{J}")
                 for J in range(4)] for mp in range(4)]
        zeros384 = const.tile([P, 384], bf16, tag="zeros384")
        nc.vector.memset(zeros384[:], 0.0)
        ones8 = const.tile([P, 8], f32, tag="ones8")
        nc.vector.memset(ones8[:], 1.0)
        ones_src = ones8[:].rearrange("p (mp h one) -> p mp h one", mp=4, h=2)
        for i in range(16):
            dst = vv[i][:].rearrange("p (mp h d) -> p mp h d", mp=4, h=2)
            nc.vector.tensor_copy(dst[:, :, :, 64:65], ones_src[:, :, :, :])

        dram = ctx.enter_context(tc.tile_pool(name="dram", bufs=1, space="DRAM"))
        rs_in = dram.tile([T, D], f32)
        rs_out = dram.tile([T // 2, D], f32)

        # ---------------- phase 1: load x, transpose, compute v ----------------
        with ExitStack() as p1:
            xload = p1.enter_context(tc.tile_pool(name="xload", bufs=5))
            xtf = p1.enter_context(tc.tile_pool(name="xtf", bufs=1))
            wvp = p1.enter_context(tc.tile_pool(name="wv", bufs=1))
            tpps = p1.enter_context(tc.tile_pool(name="tpps", bufs=2, space="PSUM"))
            vps = p1.enter_context(tc.tile_pool(name="vps", bufs=2, space="PSUM"))
            xTf = [xtf.tile([P, 512], f32r, tag=f"xTf{k}", name=f"xTf{k}") for k in range(8)]
            wv_t = [wvp.tile([P, 512], f32r, tag=f"wvt{k}", name=f"wvt{k}") for k in range(8)]
            for qq in range(4):  # t-quarters
                xi = []
                for ii in range(4):
                    xt_ = xload.tile([P, D], f32r, tag="x")
                    r0 = (qq * 4 + ii) * P
                    nc.sync.dma_start(xt_[:], x_d.ap()[r0 : r0 + P, :])
                    xi.append(xt_)
                if qq == 0:
                    # weight loads queue after the first x tiles
                    for k in range(8):
                        nc.sync.dma_start(wv_t[k][:], wv_d.ap()[k * P : (k + 1) * P, :])
                    for hp in range(4):
                        nc.sync.dma_start(
                            wproj_t[hp][:], wproj_d.ap()[hp * P : (hp + 1) * P, :]
                        )
                    _load_small_consts()
                for k in range(8):
                    tp = tpps.tile([P, 512], f32r, tag="tp")
                    for ii in range(4):
                        nc.tensor.transpose(
                            tp[:, ii * P : (ii + 1) * P],
                            xi[ii][:, k * P : (k + 1) * P],
                            ident[:],
                        )
                    # two evictions: f32 quarter copy (value path) + bf16 resident
                    nc.vector.tensor_copy(xTf[k][:], tp[:])
                    nc.scalar.copy(xT[k][:, qq * 512 : (qq + 1) * 512], tp[:])
                # v for this quarter's 4 t-tiles
                for il in range(4):
                    i = qq * 4 + il
                    ps = vps.tile([P, 512], f32, tag="vp")
                    for k in range(8):
                        nc.tensor.matmul(
                            ps[:],
                            xTf[k][:, il * P : (il + 1) * P],
                            wv_t[k][:],
                            start=(k == 0), stop=(k == 7),
                        )
                    # strided evict: psum [p, (mp h d)] d=64 -> vv [p, (mp h d65)]
                    src = ps[:].rearrange("p (mp h d) -> p mp h d", mp=4, h=2)
                    dst = vv[i][:].rearrange("p (mp h d) -> p mp h d", mp=4, h=2)
                    nc.vector.tensor_copy(dst[:, :, :, 0:64], src[:, :, :, :])
            _dbg(nc, "xTf7", xTf[7][:])
            _dbg(nc, "xT0", xT[0][:])
            _dbg(nc, "vv0", vv[0][:])

        # ---------------- phase 2: per head-pair qkv + attention ----------------
        with ExitStack() as p2:
            qkt_pool = p2.enter_context(tc.tile_pool(name="qkt", bufs=1))
            qkT = [qkt_pool.tile([P, T], f32r, tag=f"qkT{m}", name=f"qkT{m}") for m in range(8)]
            wqkp = p2.enter_context(tc.tile_pool(name="wqk", bufs=1))
            atp = p2.enter_context(tc.tile_pool(name="atp", bufs=3))
            recip = p2.enter_context(tc.tile_pool(name="recip", bufs=1))
            bcast = p2.enter_context(tc.tile_pool(name="bcast", bufs=1))
            tmpb = p2.enter_context(tc.tile_pool(name="tmpb", bufs=1))
            qkps = p2.enter_context(tc.tile_pool(name="qkps", bufs=2, space="PSUM"))
            stps = p2.enter_context(tc.tile_pool(name="stps", bufs=2, space="PSUM"))
            oups = p2.enter_context(tc.tile_pool(name="oups", bufs=1, space="PSUM"))

            for mp in range(4):
                for m in (mp, 4 + mp):
                    wq_t = []
                    for k in range(8):
                        wt = wqkp.tile([P, P], bf16, tag=f"wqkt{k}", name=f"wqkt{k}")
                        nc.sync.dma_start(
                            wt[:],
                            wqk_d.ap()[k * P : (k + 1) * P, m * P : (m + 1) * P],
                        )
                        wq_t.append(wt)
                    for n in range(4):
                        ps = qkps.tile([P, 512], f32, tag="qkp")
                        for k in range(8):
                            nc.tensor.matmul(
                                ps[:], wq_t[k][:],
                                xT[k][:, n * 512 : (n + 1) * 512],
                                start=(k == 0), stop=(k == 7),
                            )
                        nc.vector.tensor_scalar_add(
                            qkT[m][:, n * 512 : (n + 1) * 512], ps[:], bq[m][:]
                        )
                qs, ks = qkT[mp], qkT[4 + mp]
                for J in range(4):
                    nj = 4 * J + 4
                    ouA = oups.tile([65, 512], f32, tag="ouA")
                    ouB = oups.tile([65, 512], f32, tag="ouB")
                    Js = slice(J * 512, (J + 1) * 512)
                    for j in range(nj):
                        sT = stps.tile([P, 1024], f32, tag="sT")
                        js = slice(j * P, (j + 1) * P)
                        nc.tensor.matmul(
                            sT[:, 0:512],
                            ks[0:64, js], qs[0:64, Js],
                            start=True, stop=True, tile_position=(0, 0),
                        )
                        nc.tensor.matmul(
                            sT[:, 512:1024],
                            ks[64:128, js], qs[64:128, Js],
                            start=True, stop=True, tile_position=(64, 0),
                        )
                        at = atp.tile([P, 1024], f32r, tag="at")
                        i = j - 4 * J
                        if i > 0:
                            c0 = 128 * i
                            src_v = sT[:].rearrange("p (h c) -> p h c", h=2)
                            dst_v = at[:].rearrange("p (h c) -> p h c", h=2)
                            nc.scalar.activation(
                                dst_v[:, :, c0:512], src_v[:, :, c0:512],
                                EXP, bias=0.0, scale=0.125,
                            )
                        else:
                            nc.scalar.activation(at[:], sT[:], EXP, bias=0.0, scale=0.125)
                        if i >= 0:
                            # diagonal-straddling block: zero cols < 128i, apply
                            # the triangle on cols [128i, 128i+128)
                            for h0 in (0, 512):
                                c0 = h0 + 128 * i
                                if i > 0:
                                    nc.vector.tensor_copy(
                                        at[:, h0 : h0 + 128 * i],
                                        zeros384[:, 0 : 128 * i],
                                    )
                                nc.vector.tensor_mul(
                                    at[:, c0 : c0 + 128],
                                    at[:, c0 : c0 + 128], mask_tri[:],
                                )
                        if mp == 0 and J == 0 and j == 0:
                            _dbg(nc, "at000", at[:])
                        nc.tensor.matmul(
                            ouA[:], vv[j][:, 130 * mp : 130 * mp + 65],
                            at[:, 0:512],
                            start=(j == 0), stop=(j == nj - 1),
                        )
                        nc.tensor.matmul(
                            ouB[:], vv[j][:, 130 * mp + 65 : 130 * mp + 130],
                            at[:, 512:1024],
                            start=(j == 0), stop=(j == nj - 1),
                        )
                    # normalize by softmax denominator (psum row 64) and evict
                    if mp == 0 and J == 0 and _DEBUG_SINK is not None:
                        for _nm, _ou in (("ouA00", ouA), ("ouB00", ouB)):
                            if _nm in _DEBUG_SINK:
                                _dt = atp.tile([65, 512], f32, tag=f"dbg{_nm}", name=f"dbg{_nm}")
                                nc.vector.tensor_copy(_dt[:], _ou[:])
                                nc.sync.dma_start(_DEBUG_SINK[_nm].ap(), _dt[:])
                    # Lazy normalization: raw-evict values + denominators so
                    # the psum slots free in ~1us, then compute reciprocals
                    # BATCHED: a [1,1024] denom row is repacked via a DRAM
                    # round-trip into [128,8] so the DVE iterative divide runs
                    # on all lanes (0.04us) instead of one lane (5us). outN is
                    # only read by the projection, so this chain is off the
                    # attention critical path.
                    dA = recip.tile([1, 512], f32, tag="dA")
                    dB = recip.tile([1, 512], f32, tag="dB")
                    tb = tmpb.tile([64, 512], f32r, tag="tb")
                    nc.vector.tensor_copy(dA[:], ouA[64:65, :])
                    nc.vector.tensor_copy(outN[mp][J][0:64, :], ouA[0:64, :])
                    nc.vector.tensor_copy(dB[:], ouB[64:65, :])
                    nc.vector.tensor_copy(tb[:], ouB[0:64, :])
                    nc.sync.dma_start(outN[mp][J][64:128, :], tb[:])
                    eager = (mp == 3)
                    dramD = dram.tile([2, 512], f32, tag="dramD", name="dramD")
                    if eager:
                        nc.vector.reciprocal(dA[:], dA[:])
                        nc.vector.reciprocal(dB[:], dB[:])
                    else:
                        nc.sync.dma_start(dramD[0:1, :], dA[:])
                        nc.sync.dma_start(dramD[1:2, :], dB[:])
                        dPack = recip.tile([P, 8], f32, tag="dPack")
                        nc.sync.dma_start(dPack[:], dramD[:].rearrange("a (p c) -> (a p c)", p=64).rearrange("(p c) -> p c", p=P))
                        nc.vector.reciprocal(dPack[:], dPack[:])
                        nc.sync.dma_start(dramD[:].rearrange("a (p c) -> (a p c)", p=64).rearrange("(p c) -> p c", p=P), dPack[:])
                        nc.sync.dma_start(dA[:], dramD[0:1, :])
                        nc.sync.dma_start(dB[:], dramD[1:2, :])
                    bc = bcast.tile([64, 512], f32, tag="bc")
                    nc.gpsimd.partition_broadcast(bc[:, :], dA[:], channels=64)
                    bcB = bcast.tile([64, 512], f32, tag="bcB")
                    nc.gpsimd.partition_broadcast(bcB[:, :], dB[:], channels=64)
                    nc.vector.tensor_mul(outN[mp][J][0:64, :], outN[mp][J][0:64, :], bc[:, :])
                    # head B sits on partitions 64-127: broadcast lands at base
                    # 0 (HW quirk), so DMA-shift the bcast row block up.
                    bcB64 = bcast.tile([P, 512], f32, tag="bcB64")
                    nc.sync.dma_start(bcB64[64:128, :], bcB[:, :])
                    nc.vector.tensor_mul(outN[mp][J][64:128, :], outN[mp][J][64:128, :], bcB64[64:128, :])
            _dbg(nc, "qkT0", qkT[0][:])
            _dbg(nc, "qkT4", qkT[4][:])
            if _DEBUG_SINK is not None and "outN0" in _DEBUG_SINK:
                for J in range(4):
                    nc.sync.dma_start(
                        _DEBUG_SINK["outN0"].ap()[:, J * 512 : (J + 1) * 512],
                        outN[0][J][:],
                    )

            # ---- output projection (in p2 scope: fills the ACT-bound attn tail;
            #      psum reuses the idle qkv pool, evict tiles reuse atp) ----
            for i in range(16):
                for n in range(2):
                    ps = qkps.tile([P, 512], f32, tag="qkp", name="fp")
                    for hp in range(4):
                        nc.tensor.matmul(
                            ps[:],
                            outN[hp][i // 4][:, (i % 4) * P : (i % 4 + 1) * P],
                            wproj_t[hp][:, n * 512 : (n + 1) * 512],
                            start=(hp == 0), stop=(hp == 3),
                        )
                    fin = atp.tile([P, 512], f32, tag="at", name="fin")
                    nc.vector.tensor_add(fin[:], ps[:], beta_b[:, n * 512 : (n + 1) * 512])
                    nc.sync.dma_start(
                        rs_in[i * P : (i + 1) * P, n * 512 : (n + 1) * 512], fin[:]
                    )
            _dbg(nc, "rs_in", rs_in[:])

        # ---------------- ReduceScatter + output ----------------
        if globals().get("_NO_COLLECTIVE"):
            # profiling-only variant (TimelineSim is single-core)
            nc.sync.dma_start(out_d.ap(), rs_in[0 : T // 2, :])
        else:
            nc.gpsimd.collective_compute(
                "ReduceScatter", mybir.AluOpType.add,
                replica_groups=[[0, 1], [2, 3], [4, 5], [6, 7]],
                ins=[rs_in.opt()], outs=[rs_out.opt()],
            )
            nc.sync.dma_start(out_d.ap(), rs_out[:])


def _build():
    if "nc" in _CACHE:
        return _CACHE["nc"]
    nc = bacc.Bacc("TRN2", target_bir_lowering=False, debug=False, num_devices=NCORES)
    x_d = nc.dram_tensor("x", [T, D], f32r, kind="ExternalInput")
    wqk_d = nc.dram_tensor("w_qk", [D, 1024], bf16, kind="ExternalInput")
    wv_d = nc.dram_tensor("w_v", [D, 512], f32r, kind="ExternalInput")
    bqk_d = nc.dram_tensor("b_qk", [8, P, 1], f32, kind="ExternalInput")
    wproj_d = nc.dram_tensor("w_proj", [512, D], f32r, kind="ExternalInput")
    beta_d = nc.dram_tensor("beta", [1, D], bf16, kind="ExternalInput")
    out_d = nc.dram_tensor("out", [T // 2, D], f32, kind="ExternalOutput")
    with tile.TileContext(nc) as tc:
        _emit(nc, tc, x_d, wqk_d, wv_d, bqk_d, wproj_d, beta_d, out_d)
    nc.compile()
    _CACHE["nc"] = nc
    return nc


def make_in_maps(x, w_qkv, b_qkv, w_proj, b_proj):
    x = np.asarray(x, np.float32)
    w_qkv = np.asarray(w_qkv, np.float32)
    b_qkv = np.asarray(b_qkv, np.float32)
    w_proj = np.asarray(w_proj, np.float32)
    b_proj = np.asarray(b_proj, np.float32)
    in_maps = []
    for c in range(NCORES):
        b, g = c // 2, c % 2
        qcols = slice(g * 512, (g + 1) * 512)
        kcols = slice(D + g * 512, D + (g + 1) * 512)
        vcols = slice(2 * D + g * 512, 2 * D + (g + 1) * 512)
        w_qk = np.concatenate([w_qkv[:, qcols], w_qkv[:, kcols]], axis=1)
        b_qk = np.concatenate([b_qkv[qcols], b_qkv[kcols]])
        wp = np.ascontiguousarray(w_proj[g * 512 : (g + 1) * 512, :])
        beta = wp.T @ b_qkv[vcols]
        if g == 0:
            beta = beta + b_proj
        in_maps.append({
            "x": np.ascontiguousarray(x[b]),
            "w_qk": np.ascontiguousarray(w_qk).astype(ml_dtypes.bfloat16),
            "w_v": np.ascontiguousarray(w_qkv[:, vcols]),
            "b_qk": b_qk.reshape(8, P, 1),
            "w_proj": wp,
            "beta": beta.reshape(1, D).astype(ml_dtypes.bfloat16),
        })
    return in_maps


def kernel(x, w_qkv, b_qkv, w_proj, b_proj, trace=False, **run_kwargs):
    global LAST_RESULTS
    nc = _build()
    in_maps = make_in_maps(x, w_qkv, b_qkv, w_proj, b_proj)
    res = run_bass_kernel_spmd(
        nc, in_maps, core_ids=list(range(NCORES)), trace=trace, **run_kwargs
    )
    LAST_RESULTS = res
    out = np.empty((B, T, D), np.float32)
    for b in range(B):
        out[b, : T // 2] = res.results[2 * b]["out"]
        out[b, T // 2 :] = res.results[2 * b + 1]["out"]
    return out



# revision 3
# speedup vs baseline: 1.2927x; 1.2927x over previous
"""Causal self-attention Bass kernel for 8 trn2 NeuronCores.

Problem: B=4, T=2048, D=1024, H=16 causal self-attention (qkv proj + attn + out proj).

Sharding: core c = 2*b + g handles batch b (=c//2) and head-group g (=c%2, 8 heads).
Per core:
  - qkv projection column-shard: q,k,v columns for its 8 heads only.
  - flash-style attention in transposed-score layout sT[tk, tq]; softmax denominator
    via an extra ones-column in the AV matmul (row 64 of the [65, 512] psum output).
  - output projection row-shard (w_proj rows for its head dims) -> partial [T, D].
  - pairwise ReduceScatter {2b, 2b+1} sums the two head-group partials and splits
    output rows t: even core -> rows [0,1024), odd -> [1024, 2048).
Host reassembles by stacking the two halves per batch.

Precision: bf16 operands everywhere on the matmul paths (1 cyc/row streaming, FWL
weight loads); psum accumulation stays f32. The softmax is shift-robust so bf16
score inputs cost only absolute O(1e-2) score error. The pairwise ReduceScatter
runs in bf16 (halves collective bytes); host converts the bf16 output to f32.
b_v is folded into beta = b_proj(once per pair) + w_proj_shard.T @ b_v_shard since
softmax rows sum to 1.

Diagonal-block handling: score tiles that straddle the causal diagonal get a
partial exp (cols >= 128*i only) and ONE strided triangle-mask multiply covering
both heads; the AV matmuls then read only the valid column range, so no zero-fill
of the masked region is needed.
"""

from contextlib import ExitStack

import ml_dtypes
import numpy as np

import concourse.bass as bass
import concourse.mybir as mybir
import concourse.tile as tile
from concourse import bacc
from concourse.bass_utils import run_bass_kernel_spmd

B, T, D, H = 4, 2048, 1024, 16
HD = D // H  # 64
NCORES = 8
P = 128
f32 = mybir.dt.float32
f32r = mybir.dt.float32r
bf16 = mybir.dt.bfloat16
EXP = mybir.ActivationFunctionType.Exp

_CACHE = {}
LAST_RESULTS = None
_DEBUG_SINK = None


def _dbg(nc, name, ap):
    if _DEBUG_SINK is not None and name in _DEBUG_SINK:
        nc.sync.dma_start(_DEBUG_SINK[name].ap(), ap)


def _emit(nc, tc, x_d, wqk_d, wv_d, bqk_d, wproj_d, beta_d, out_d):
    with ExitStack() as ctx:
        # ---------------- constants / persistent tiles ----------------
        const = ctx.enter_context(tc.tile_pool(name="const", bufs=1))
        bootc = ctx.enter_context(tc.tile_pool(name="boot", bufs=1))
        ident_f = bootc.tile([P, P], bf16, tag="ident_f")
        nc.gpsimd.memset(ident_f[:], 0.0)
        nc.gpsimd.affine_select(
            out=ident_f[:], in_=ident_f[:],
            compare_op=mybir.AluOpType.not_equal, fill=1.0,
            base=0, pattern=[[-1, P]], channel_multiplier=1,
        )
        # triangle mask [128,256] = two copies of (keep iff f >= p)
        mask_tri2 = const.tile([P, 2 * P], bf16, tag="mask_tri2")
        nc.gpsimd.memset(mask_tri2[:], 1.0)
        for h0 in (0, P):
            nc.gpsimd.affine_select(
                out=mask_tri2[:, h0 : h0 + P], in_=mask_tri2[:, h0 : h0 + P],
                compare_op=mybir.AluOpType.is_ge, fill=0.0,
                base=0, pattern=[[1, P]], channel_multiplier=-1,
            )
        bq = [const.tile([P, 1], f32, tag=f"bq{m}", name=f"bq{m}") for m in range(8)]
        beta_b = const.tile([P, D], bf16, tag="beta_b")

        def _load_small_consts():
            for m in range(8):
                nc.sync.dma_start(bq[m][:], bqk_d.ap()[m])
            nc.sync.dma_start(beta_b[0:1, :], beta_d.ap())
            nc.gpsimd.partition_broadcast(beta_b[:], beta_b[0:1, :], channels=P)
        # w_proj pool reserved here; its DMAs are emitted after phase 1 starts
        # so the x loads win the DMA queue.
        wpp = ctx.enter_context(tc.tile_pool(name="wpp", bufs=1))
        wproj_t = [wpp.tile([P, D], bf16, tag=f"wp{hp}", name=f"wp{hp}") for hp in range(4)]
        _dbg(nc, "beta_b", beta_b[:])

        # persistent activations
        xt_pool = ctx.enter_context(tc.tile_pool(name="xt", bufs=1))
        xT = [xt_pool.tile([P, T], bf16, tag=f"xT{k}", name=f"xT{k}") for k in range(8)]
        vv_pool = ctx.enter_context(tc.tile_pool(name="vv", bufs=1))
        vv = [vv_pool.tile([P, 520], bf16, tag=f"vv{i}", name=f"vv{i}") for i in range(16)]
        on_pool = ctx.enter_context(tc.tile_pool(name="outn", bufs=1))
        outN = [[on_pool.tile([P, 512], bf16, tag=f"outN{mp}J{J}", name=f"outN{mp}J{J}")
                 for J in range(4)] for mp in range(4)]
        ones8 = const.tile([P, 8], bf16, tag="ones8")
        nc.vector.memset(ones8[:], 1.0)
        ones_src = ones8[:].rearrange("p (mp h one) -> p mp h one", mp=4, h=2)
        for i in range(16):
            dst = vv[i][:].rearrange("p (mp h d) -> p mp h d", mp=4, h=2)
            nc.vector.tensor_copy(dst[:, :, :, 64:65], ones_src[:, :, :, :])

        dram = ctx.enter_context(tc.tile_pool(name="dram", bufs=1, space="DRAM"))
        rs_in = dram.tile([T, D], bf16)
        rs_out = dram.tile([T // 2, D], bf16)

        # ---------------- phase 1: load x, transpose, compute v ----------------
        with ExitStack() as p1:
            xload = p1.enter_context(tc.tile_pool(name="xload", bufs=5))
            wvp = p1.enter_context(tc.tile_pool(name="wv", bufs=1))
            tpps = p1.enter_context(tc.tile_pool(name="tpps", bufs=2, space="PSUM"))
            vps = p1.enter_context(tc.tile_pool(name="vps", bufs=2, space="PSUM"))
            wv_t = [wvp.tile([P, 512], bf16, tag=f"wvt{k}", name=f"wvt{k}") for k in range(8)]
            for qq in range(4):  # t-quarters
                xi = []
                for ii in range(4):
                    xt_ = xload.tile([P, D], bf16, tag="x")
                    r0 = (qq * 4 + ii) * P
                    nc.sync.dma_start(xt_[:], x_d.ap()[r0 : r0 + P, :])
                    xi.append(xt_)
                if qq == 0:
                    # weight loads queue after the first x tiles
                    for k in range(8):
                        nc.sync.dma_start(wv_t[k][:], wv_d.ap()[k * P : (k + 1) * P, :])
                    for hp in range(4):
                        nc.sync.dma_start(
                            wproj_t[hp][:], wproj_d.ap()[hp * P : (hp + 1) * P, :]
                        )
                    _load_small_consts()
                for k in range(8):
                    tp = tpps.tile([P, 512], bf16, tag="tp")
                    for ii in range(4):
                        nc.tensor.transpose(
                            tp[:, ii * P : (ii + 1) * P],
                            xi[ii][:, k * P : (k + 1) * P],
                            ident_f[:],
                        )
                    nc.scalar.copy(xT[k][:, qq * 512 : (qq + 1) * 512], tp[:])
                # v for this quarter's 4 t-tiles
                for il in range(4):
                    i = qq * 4 + il
                    ps = vps.tile([P, 512], f32, tag="vp")
                    for k in range(8):
                        nc.tensor.matmul(
                            ps[:],
                            xT[k][:, i * P : (i + 1) * P],
                            wv_t[k][:],
                            start=(k == 0), stop=(k == 7),
                        )
                    # strided evict: psum [p, (mp h d)] d=64 -> vv [p, (mp h d65)]
                    src = ps[:].rearrange("p (mp h d) -> p mp h d", mp=4, h=2)
                    dst = vv[i][:].rearrange("p (mp h d) -> p mp h d", mp=4, h=2)
                    nc.vector.tensor_copy(dst[:, :, :, 0:64], src[:, :, :, :])
            _dbg(nc, "xT0", xT[0][:])
            _dbg(nc, "vv0", vv[0][:])

        # ---------------- phase 2: per head-pair qkv + attention ----------------
        with ExitStack() as p2:
            qkt_pool = p2.enter_context(tc.tile_pool(name="qkt", bufs=1))
            qkT = [qkt_pool.tile([P, T], bf16, tag=f"qkT{m}", name=f"qkT{m}") for m in range(8)]
            wqkp = p2.enter_context(tc.tile_pool(name="wqk", bufs=1))
            atp = p2.enter_context(tc.tile_pool(name="atp", bufs=3))
            recip = p2.enter_context(tc.tile_pool(name="recip", bufs=1))
            bcast = p2.enter_context(tc.tile_pool(name="bcast", bufs=1))
            tmpb = p2.enter_context(tc.tile_pool(name="tmpb", bufs=1))
            qkps = p2.enter_context(tc.tile_pool(name="qkps", bufs=2, space="PSUM"))
            stps = p2.enter_context(tc.tile_pool(name="stps", bufs=2, space="PSUM"))
            oups = p2.enter_context(tc.tile_pool(name="oups", bufs=1, space="PSUM"))

            for mp in range(4):
                for m in (mp, 4 + mp):
                    wq_t = []
                    for k in range(8):
                        wt = wqkp.tile([P, P], bf16, tag=f"wqkt{k}", name=f"wqkt{k}")
                        nc.sync.dma_start(
                            wt[:],
                            wqk_d.ap()[k * P : (k + 1) * P, m * P : (m + 1) * P],
                        )
                        wq_t.append(wt)
                    for n in range(4):
                        ps = qkps.tile([P, 512], f32, tag="qkp")
                        for k in range(8):
                            nc.tensor.matmul(
                                ps[:], wq_t[k][:],
                                xT[k][:, n * 512 : (n + 1) * 512],
                                start=(k == 0), stop=(k == 7),
                            )
                        nc.vector.tensor_scalar_add(
                            qkT[m][:, n * 512 : (n + 1) * 512], ps[:], bq[m][:]
                        )
                qs, ks = qkT[mp], qkT[4 + mp]
                for J in range(4):
                    nj = 4 * J + 4
                    ouA = oups.tile([65, 512], f32, tag="ouA")
                    ouB = oups.tile([65, 512], f32, tag="ouB")
                    Js = slice(J * 512, (J + 1) * 512)
                    for j in range(nj):
                        sT = stps.tile([P, 1024], f32, tag="sT")
                        js = slice(j * P, (j + 1) * P)
                        nc.tensor.matmul(
                            sT[:, 0:512],
                            ks[0:64, js], qs[0:64, Js],
                            start=True, stop=True, tile_position=(0, 0),
                        )
                        nc.tensor.matmul(
                            sT[:, 512:1024],
                            ks[64:128, js], qs[64:128, Js],
                            start=True, stop=True, tile_position=(64, 0),
                        )
                        at = atp.tile([P, 1024], bf16, tag="at")
                        i = j - 4 * J
                        c0 = 128 * i if i > 0 else 0
                        if i > 0:
                            src_v = sT[:].rearrange("p (h c) -> p h c", h=2)
                            dst_v = at[:].rearrange("p (h c) -> p h c", h=2)
                            nc.scalar.activation(
                                dst_v[:, :, c0:512], src_v[:, :, c0:512],
                                EXP, bias=0.0, scale=0.125,
                            )
                        else:
                            nc.scalar.activation(at[:], sT[:], EXP, bias=0.0, scale=0.125)
                        if i >= 0:
                            # diagonal-straddling block: one strided multiply
                            # applies the triangle to both heads' [c0,c0+128)
                            atv = at[:].rearrange("p (h c) -> p h c", h=2)
                            mkv = mask_tri2[:].rearrange("p (h c) -> p h c", h=2)
                            nc.vector.tensor_mul(
                                atv[:, :, c0 : c0 + 128],
                                atv[:, :, c0 : c0 + 128],
                                mkv[:, :, :],
                            )
                        if mp == 0 and J == 0 and j == 0:
                            _dbg(nc, "at000", at[:])
                        nc.tensor.matmul(
                            ouA[:, c0:512], vv[j][:, 130 * mp : 130 * mp + 65],
                            at[:, c0:512],
                            start=(j == 0), stop=(j == nj - 1),
                            skip_group_check=(i > 0 or j == nj - 1),
                        )
                        nc.tensor.matmul(
                            ouB[:, c0:512], vv[j][:, 130 * mp + 65 : 130 * mp + 130],
                            at[:, 512 + c0 : 1024],
                            start=(j == 0), stop=(j == nj - 1),
                            skip_group_check=(i > 0 or j == nj - 1),
                        )
                    # normalize by softmax denominator (psum row 64) and evict
                    if mp == 0 and J == 0 and _DEBUG_SINK is not None:
                        for _nm, _ou in (("ouA00", ouA), ("ouB00", ouB)):
                            if _nm in _DEBUG_SINK:
                                _dt = atp.tile([65, 512], f32, tag=f"dbg{_nm}", name=f"dbg{_nm}")
                                nc.vector.tensor_copy(_dt[:], _ou[:])
                                nc.sync.dma_start(_DEBUG_SINK[_nm].ap(), _dt[:])
                    # Lazy normalization: raw-evict values + denominators so
                    # the psum slots free in ~1us, then compute reciprocals
                    # BATCHED: a [1,1024] denom row is repacked via a DRAM
                    # round-trip into [128,8] so the DVE iterative divide runs
                    # on all lanes (0.04us) instead of one lane (5us). outN is
                    # only read by the projection, so this chain is off the
                    # attention critical path.
                    dA = recip.tile([1, 512], f32, tag="dA")
                    dB = recip.tile([1, 512], f32, tag="dB")
                    tb = tmpb.tile([64, 512], bf16, tag="tb")
                    nc.vector.tensor_copy(dA[:], ouA[64:65, :])
                    nc.vector.tensor_copy(outN[mp][J][0:64, :], ouA[0:64, :])
                    nc.vector.tensor_copy(dB[:], ouB[64:65, :])
                    nc.vector.tensor_copy(tb[:], ouB[0:64, :])
                    nc.sync.dma_start(outN[mp][J][64:128, :], tb[:])
                    eager = (mp == 3)
                    dramD = dram.tile([2, 512], f32, tag="dramD", name="dramD")
                    if eager:
                        nc.vector.reciprocal(dA[:], dA[:])
                        nc.vector.reciprocal(dB[:], dB[:])
                    else:
                        nc.sync.dma_start(dramD[0:1, :], dA[:])
                        nc.sync.dma_start(dramD[1:2, :], dB[:])
                        dPack = recip.tile([P, 8], f32, tag="dPack")
                        nc.sync.dma_start(dPack[:], dramD[:].rearrange("a (p c) -> (a p c)", p=64).rearrange("(p c) -> p c", p=P))
                        nc.vector.reciprocal(dPack[:], dPack[:])
                        nc.sync.dma_start(dramD[:].rearrange("a (p c) -> (a p c)", p=64).rearrange("(p c) -> p c", p=P), dPack[:])
                        nc.sync.dma_start(dA[:], dramD[0:1, :])
                        nc.sync.dma_start(dB[:], dramD[1:2, :])
                    bc = bcast.tile([64, 512], f32, tag="bc")
                    nc.gpsimd.partition_broadcast(bc[:, :], dA[:], channels=64)
                    bcB = bcast.tile([64, 512], f32, tag="bcB")
                    nc.gpsimd.partition_broadcast(bcB[:, :], dB[:], channels=64)
                    nc.vector.tensor_mul(outN[mp][J][0:64, :], outN[mp][J][0:64, :], bc[:, :])
                    # head B sits on partitions 64-127: broadcast lands at base
                    # 0 (HW quirk), so DMA-shift the bcast row block up.
                    bcB64 = bcast.tile([P, 512], f32, tag="bcB64")
                    nc.sync.dma_start(bcB64[64:128, :], bcB[:, :])
                    nc.vector.tensor_mul(outN[mp][J][64:128, :], outN[mp][J][64:128, :], bcB64[64:128, :])
            _dbg(nc, "qkT0", qkT[0][:])
            _dbg(nc, "qkT4", qkT[4][:])
            if _DEBUG_SINK is not None and "outN0" in _DEBUG_SINK:
                for J in range(4):
                    nc.sync.dma_start(
                        _DEBUG_SINK["outN0"].ap()[:, J * 512 : (J + 1) * 512],
                        outN[0][J][:],
                    )

            # ---- output projection (in p2 scope: fills the ACT-bound attn tail;
            #      psum reuses the idle qkv pool, evict tiles reuse atp) ----
            for i in range(16):
                for n in range(2):
                    ps = qkps.tile([P, 512], f32, tag="qkp", name="fp")
                    for hp in range(4):
                        nc.tensor.matmul(
                            ps[:],
                            outN[hp][i // 4][:, (i % 4) * P : (i % 4 + 1) * P],
                            wproj_t[hp][:, n * 512 : (n + 1) * 512],
                            start=(hp == 0), stop=(hp == 3),
                        )
                    fin = atp.tile([P, 512], bf16, tag="fin")
                    nc.vector.tensor_add(fin[:], ps[:], beta_b[:, n * 512 : (n + 1) * 512])
                    nc.sync.dma_start(
                        rs_in[i * P : (i + 1) * P, n * 512 : (n + 1) * 512], fin[:]
                    )
            _dbg(nc, "rs_in", rs_in[:])

        # ---------------- ReduceScatter + output ----------------
        if globals().get("_NO_COLLECTIVE"):
            # profiling-only variant (TimelineSim is single-core)
            nc.sync.dma_start(out_d.ap(), rs_in[0 : T // 2, :])
        else:
            nc.gpsimd.collective_compute(
                "ReduceScatter", mybir.AluOpType.add,
                replica_groups=[[0, 1], [2, 3], [4, 5], [6, 7]],
                ins=[rs_in.opt()], outs=[rs_out.opt()],
            )
            nc.sync.dma_start(out_d.ap(), rs_out[:])


def _build():
    if "nc" in _CACHE:
        return _CACHE["nc"]
    nc = bacc.Bacc("TRN2", target_bir_lowering=False, debug=False, num_devices=NCORES)
    x_d = nc.dram_tensor("x", [T, D], bf16, kind="ExternalInput")
    wqk_d = nc.dram_tensor("w_qk", [D, 1024], bf16, kind="ExternalInput")
    wv_d = nc.dram_tensor("w_v", [D, 512], bf16, kind="ExternalInput")
    bqk_d = nc.dram_tensor("b_qk", [8, P, 1], f32, kind="ExternalInput")
    wproj_d = nc.dram_tensor("w_proj", [512, D], bf16, kind="ExternalInput")
    beta_d = nc.dram_tensor("beta", [1, D], bf16, kind="ExternalInput")
    out_d = nc.dram_tensor("out", [T // 2, D], bf16, kind="ExternalOutput")
    with tile.TileContext(nc) as tc:
        _emit(nc, tc, x_d, wqk_d, wv_d, bqk_d, wproj_d, beta_d, out_d)
    nc.compile()
    _CACHE["nc"] = nc
    return nc


def make_in_maps(x, w_qkv, b_qkv, w_proj, b_proj):
    x = np.asarray(x, np.float32)
    w_qkv = np.asarray(w_qkv, np.float32)
    b_qkv = np.asarray(b_qkv, np.float32)
    w_proj = np.asarray(w_proj, np.float32)
    b_proj = np.asarray(b_proj, np.float32)
    in_maps = []
    for c in range(NCORES):
        b, g = c // 2, c % 2
        qcols = slice(g * 512, (g + 1) * 512)
        kcols = slice(D + g * 512, D + (g + 1) * 512)
        vcols = slice(2 * D + g * 512, 2 * D + (g + 1) * 512)
        w_qk = np.concatenate([w_qkv[:, qcols], w_qkv[:, kcols]], axis=1)
        b_qk = np.concatenate([b_qkv[qcols], b_qkv[kcols]])
        wp = np.ascontiguousarray(w_proj[g * 512 : (g + 1) * 512, :])
        beta = wp.T @ b_qkv[vcols]
        if g == 0:
            beta = beta + b_proj
        in_maps.append({
            "x": np.ascontiguousarray(x[b]).astype(ml_dtypes.bfloat16),
            "w_qk": np.ascontiguousarray(w_qk).astype(ml_dtypes.bfloat16),
            "w_v": np.ascontiguousarray(w_qkv[:, vcols]).astype(ml_dtypes.bfloat16),
            "b_qk": b_qk.reshape(8, P, 1),
            "w_proj": wp.astype(ml_dtypes.bfloat16),
            "beta": beta.reshape(1, D).astype(ml_dtypes.bfloat16),
        })
    return in_maps


def kernel(x, w_qkv, b_qkv, w_proj, b_proj, trace=False, **run_kwargs):
    global LAST_RESULTS
    nc = _build()
    in_maps = make_in_maps(x, w_qkv, b_qkv, w_proj, b_proj)
    res = run_bass_kernel_spmd(
        nc, in_maps, core_ids=list(range(NCORES)), trace=trace, **run_kwargs
    )
    LAST_RESULTS = res
    out = np.empty((B, T, D), np.float32)
    for b in range(B):
        out[b, : T // 2] = np.asarray(res.results[2 * b]["out"], np.float32)
        out[b, T // 2 :] = np.asarray(res.results[2 * b + 1]["out"], np.float32)
    return out


# revision 4
# speedup vs baseline: 1.4040x; 1.0861x over previous
"""Causal self-attention Bass kernel for 8 trn2 NeuronCores.

Problem: B=4, T=2048, D=1024, H=16 causal self-attention (qkv proj + attn + out proj).

Sharding: core c = 2*b + g handles batch b (=c//2) and head-group g (=c%2, 8 heads).
Per core:
  - qkv projection column-shard: q,k,v columns for its 8 heads only.
  - flash-style attention in transposed-score layout sT[tk, tq]; softmax denominator
    via an extra ones-column in the AV matmul (row 64 of the [65, 512] psum output).
  - output projection row-shard (w_proj rows for its head dims) -> partial [T, D].
  - pairwise ReduceScatter {2b, 2b+1} sums the two head-group partials; chunked
    over 4 query blocks so the collective overlaps the attention/projection
    stream.  Even core ends with global rows {512J..512J+256}, odd core with
    {512J+256..512J+512}; host reassembles.

Pipeline: phase 2 runs query-block-outer (J = 0..3).  Step J computes attention
for all 4 head-pairs at query block J, while interleaving (as PE filler work
between ACT-gated attention iterations) the qk-projection for column block J+1
and the output projection + ReduceScatter chunk for block J-1.  The AV matmul is
software-pipelined one iteration behind the score matmul so the PE never waits
on the exp/mask chain.

Precision: bf16 operands everywhere on the matmul paths (1 cyc/row streaming,
FWL weight loads); psum accumulation stays f32.  The ReduceScatter runs in bf16;
host converts the output to f32.  b_v is folded into beta = b_proj(once per
pair) + w_proj_shard.T @ b_v_shard since softmax rows sum to 1.

Diagonal blocks: partial exp (cols >= 128*i only) and ONE strided triangle-mask
multiply covering both heads; AV matmuls read only the valid column range, so no
zero-fill of the masked region is needed.
"""

from contextlib import ExitStack

import ml_dtypes
import numpy as np

import concourse.bass as bass
import concourse.mybir as mybir
import concourse.tile as tile
from concourse import bacc
from concourse.bass_utils import run_bass_kernel_spmd

B, T, D, H = 4, 2048, 1024, 16
HD = D // H  # 64
NCORES = 8
P = 128
f32 = mybir.dt.float32
f32r = mybir.dt.float32r
bf16 = mybir.dt.bfloat16
EXP = mybir.ActivationFunctionType.Exp

_CACHE = {}
LAST_RESULTS = None
_DEBUG_SINK = None


def _dbg(nc, name, ap):
    if _DEBUG_SINK is not None and name in _DEBUG_SINK:
        nc.sync.dma_start(_DEBUG_SINK[name].ap(), ap)


def _emit(nc, tc, x_d, wqk_d, wv_d, bqk_d, wproj_d, beta_d, out_d):
    with ExitStack() as ctx:
        # ---------------- constants / persistent tiles ----------------
        const = ctx.enter_context(tc.tile_pool(name="const", bufs=1))
        bootc = ctx.enter_context(tc.tile_pool(name="boot", bufs=1))
        ident_f = bootc.tile([P, P], bf16, tag="ident_f")
        nc.gpsimd.memset(ident_f[:], 0.0)
        nc.gpsimd.affine_select(
            out=ident_f[:], in_=ident_f[:],
            compare_op=mybir.AluOpType.not_equal, fill=1.0,
            base=0, pattern=[[-1, P]], channel_multiplier=1,
        )
        # triangle mask [128,256] = two copies of (keep iff f >= p)
        mask_tri2 = const.tile([P, 2 * P], bf16, tag="mask_tri2")
        nc.gpsimd.memset(mask_tri2[:], 1.0)
        for h0 in (0, P):
            nc.gpsimd.affine_select(
                out=mask_tri2[:, h0 : h0 + P], in_=mask_tri2[:, h0 : h0 + P],
                compare_op=mybir.AluOpType.is_ge, fill=0.0,
                base=0, pattern=[[1, P]], channel_multiplier=-1,
            )
        bq = [const.tile([P, 1], f32, tag=f"bq{m}", name=f"bq{m}") for m in range(8)]
        beta_b = const.tile([P, D], bf16, tag="beta_b")

        def _load_small_consts():
            for m in range(8):
                nc.sync.dma_start(bq[m][:], bqk_d.ap()[m])
            nc.sync.dma_start(beta_b[0:1, :], beta_d.ap())
            nc.gpsimd.partition_broadcast(beta_b[:], beta_b[0:1, :], channels=P)
        # weight pools reserved here; DMAs are emitted after phase 1 starts
        # so the x loads win the DMA queue.
        wpp = ctx.enter_context(tc.tile_pool(name="wpp", bufs=1))
        wproj_t = [wpp.tile([P, D], bf16, tag=f"wp{hp}", name=f"wp{hp}") for hp in range(4)]
        wqkp = ctx.enter_context(tc.tile_pool(name="wqk", bufs=1))
        wq_t = [[wqkp.tile([P, P], bf16, tag=f"wqkt{m}k{k}", name=f"wqkt{m}k{k}")
                 for k in range(8)] for m in range(8)]
        _dbg(nc, "beta_b", beta_b[:])

        # persistent activations
        xt_pool = ctx.enter_context(tc.tile_pool(name="xt", bufs=1))
        xT = [xt_pool.tile([P, T], bf16, tag=f"xT{k}", name=f"xT{k}") for k in range(8)]
        vv_pool = ctx.enter_context(tc.tile_pool(name="vv", bufs=1))
        vv = [vv_pool.tile([P, 520], bf16, tag=f"vv{i}", name=f"vv{i}") for i in range(16)]
        on_pool = ctx.enter_context(tc.tile_pool(name="outn", bufs=1))
        outN = [[on_pool.tile([P, 512], bf16, tag=f"outN{mp}J{J}", name=f"outN{mp}J{J}")
                 for J in range(4)] for mp in range(4)]
        ones8 = const.tile([P, 8], bf16, tag="ones8")
        nc.vector.memset(ones8[:], 1.0)
        ones_src = ones8[:].rearrange("p (mp h one) -> p mp h one", mp=4, h=2)
        for i in range(16):
            dst = vv[i][:].rearrange("p (mp h d) -> p mp h d", mp=4, h=2)
            nc.vector.tensor_copy(dst[:, :, :, 64:65], ones_src[:, :, :, :])

        dram = ctx.enter_context(tc.tile_pool(name="dram", bufs=1, space="DRAM"))
        rs_in = dram.tile([T, D], bf16)
        rs_out = dram.tile([T // 2, D], bf16)
        dram2 = ctx.enter_context(tc.tile_pool(name="dram2", bufs=2, space="DRAM"))

        # ---------------- phase 1: load x, transpose, compute v ----------------
        with ExitStack() as p1:
            xload = p1.enter_context(tc.tile_pool(name="xload", bufs=5))
            wvp = p1.enter_context(tc.tile_pool(name="wv", bufs=1))
            tpps = p1.enter_context(tc.tile_pool(name="tpps", bufs=2, space="PSUM"))
            vps = p1.enter_context(tc.tile_pool(name="vps", bufs=2, space="PSUM"))
            wv_t = [wvp.tile([P, 512], bf16, tag=f"wvt{k}", name=f"wvt{k}") for k in range(8)]
            for qq in range(4):  # t-quarters
                xi = []
                for ii in range(4):
                    xt_ = xload.tile([P, D], bf16, tag="x")
                    r0 = (qq * 4 + ii) * P
                    nc.sync.dma_start(xt_[:], x_d.ap()[r0 : r0 + P, :])
                    xi.append(xt_)
                if qq == 0:
                    # weight loads queue after the first x tiles
                    for k in range(8):
                        nc.sync.dma_start(wv_t[k][:], wv_d.ap()[k * P : (k + 1) * P, :])
                    for hp in range(4):
                        nc.sync.dma_start(
                            wproj_t[hp][:], wproj_d.ap()[hp * P : (hp + 1) * P, :]
                        )
                    _load_small_consts()
                if qq == 1:
                    for m in range(8):
                        for k in range(8):
                            nc.sync.dma_start(
                                wq_t[m][k][:],
                                wqk_d.ap()[k * P : (k + 1) * P, m * P : (m + 1) * P],
                            )
                for k in range(8):
                    tp = tpps.tile([P, 512], bf16, tag="tp")
                    for ii in range(4):
                        nc.tensor.transpose(
                            tp[:, ii * P : (ii + 1) * P],
                            xi[ii][:, k * P : (k + 1) * P],
                            ident_f[:],
                        )
                    nc.scalar.copy(xT[k][:, qq * 512 : (qq + 1) * 512], tp[:])
                # v for this quarter's 4 t-tiles
                for il in range(4):
                    i = qq * 4 + il
                    ps = vps.tile([P, 512], f32, tag="vp")
                    for k in range(8):
                        nc.tensor.matmul(
                            ps[:],
                            xT[k][:, i * P : (i + 1) * P],
                            wv_t[k][:],
                            start=(k == 0), stop=(k == 7),
                        )
                    # strided evict: psum [p, (mp h d)] d=64 -> vv [p, (mp h d65)]
                    src = ps[:].rearrange("p (mp h d) -> p mp h d", mp=4, h=2)
                    dst = vv[i][:].rearrange("p (mp h d) -> p mp h d", mp=4, h=2)
                    nc.vector.tensor_copy(dst[:, :, :, 0:64], src[:, :, :, :])
            _dbg(nc, "xT0", xT[0][:])
            _dbg(nc, "vv0", vv[0][:])

        # ---------------- phase 2: J-outer attention pipeline ----------------
        with ExitStack() as p2:
            qkt_pool = p2.enter_context(tc.tile_pool(name="qkt", bufs=1))
            qkT = [qkt_pool.tile([P, T], bf16, tag=f"qkT{m}", name=f"qkT{m}") for m in range(8)]
            atp = p2.enter_context(tc.tile_pool(name="atp", bufs=3))
            recip = p2.enter_context(tc.tile_pool(name="recip", bufs=2))
            bcast = p2.enter_context(tc.tile_pool(name="bcast", bufs=2))
            tmpb = p2.enter_context(tc.tile_pool(name="tmpb", bufs=2))
            qkps = p2.enter_context(tc.tile_pool(name="qkps", bufs=2, space="PSUM"))
            stps = p2.enter_context(tc.tile_pool(name="stps", bufs=2, space="PSUM"))
            oups = p2.enter_context(tc.tile_pool(name="oups", bufs=1, space="PSUM"))

            def qkproj_group(m, n):
                ps = qkps.tile([P, 512], f32, tag="qkp")
                for k in range(8):
                    nc.tensor.matmul(
                        ps[:], wq_t[m][k][:],
                        xT[k][:, n * 512 : (n + 1) * 512],
                        start=(k == 0), stop=(k == 7),
                    )
                nc.vector.tensor_scalar_add(
                    qkT[m][:, n * 512 : (n + 1) * 512], ps[:], bq[m][:]
                )

            def outproj_group(J, g):
                i = 4 * J + g // 2
                n = g % 2
                ps = qkps.tile([P, 512], f32, tag="qkp", name="fp")
                for hp in range(4):
                    nc.tensor.matmul(
                        ps[:],
                        outN[hp][J][:, (i % 4) * P : (i % 4 + 1) * P],
                        wproj_t[hp][:, n * 512 : (n + 1) * 512],
                        start=(hp == 0), stop=(hp == 3),
                    )
                fin = atp.tile([P, 512], bf16, tag="fin")
                nc.vector.tensor_add(fin[:], ps[:], beta_b[:, n * 512 : (n + 1) * 512])
                nc.sync.dma_start(
                    rs_in[i * P : (i + 1) * P, n * 512 : (n + 1) * 512], fin[:]
                )

            def rs_chunk(J):
                if globals().get("_NO_COLLECTIVE"):
                    nc.sync.dma_start(
                        out_d.ap()[256 * J : 256 * J + 256, :],
                        rs_in[512 * J : 512 * J + 256, :],
                    )
                    return
                nc.gpsimd.collective_compute(
                    "ReduceScatter", mybir.AluOpType.add,
                    replica_groups=[[0, 1], [2, 3], [4, 5], [6, 7]],
                    ins=[rs_in[512 * J : 512 * (J + 1), :].opt()],
                    outs=[rs_out[256 * J : 256 * (J + 1), :].opt()],
                )
                nc.sync.dma_start(
                    out_d.ap()[256 * J : 256 * (J + 1), :],
                    rs_out[256 * J : 256 * (J + 1), :],
                )

            def norm_chain(mp, J):
                # Lazy normalization: raw-evict values + denominators so the
                # psum slots free fast, then reciprocals via a DRAM repack
                # ([2,512] -> [128,8]) so the DVE iterative divide runs on all
                # lanes.  outN is only read by the (one-step-later) projection,
                # so this chain is off the attention critical path.
                ouA, ouB = ou_tiles[mp]
                dA = recip.tile([1, 512], f32, tag="dA")
                dB = recip.tile([1, 512], f32, tag="dB")
                tb = tmpb.tile([64, 512], bf16, tag="tb")
                nc.vector.tensor_copy(dA[:], ouA[64:65, :])
                nc.vector.tensor_copy(outN[mp][J][0:64, :], ouA[0:64, :])
                nc.vector.tensor_copy(dB[:], ouB[64:65, :])
                nc.vector.tensor_copy(tb[:], ouB[0:64, :])
                nc.sync.dma_start(outN[mp][J][64:128, :], tb[:])
                dramD = dram2.tile([2, 512], f32, tag="dramD", name="dramD")
                nc.sync.dma_start(dramD[0:1, :], dA[:])
                nc.sync.dma_start(dramD[1:2, :], dB[:])
                dPack = recip.tile([P, 8], f32, tag="dPack")
                nc.sync.dma_start(dPack[:], dramD[:].rearrange("a (p c) -> (a p c)", p=64).rearrange("(p c) -> p c", p=P))
                nc.vector.reciprocal(dPack[:], dPack[:])
                nc.sync.dma_start(dramD[:].rearrange("a (p c) -> (a p c)", p=64).rearrange("(p c) -> p c", p=P), dPack[:])
                nc.sync.dma_start(dA[:], dramD[0:1, :])
                nc.sync.dma_start(dB[:], dramD[1:2, :])
                bc = bcast.tile([64, 512], f32, tag="bc")
                nc.gpsimd.partition_broadcast(bc[:, :], dA[:], channels=64)
                bcB = bcast.tile([64, 512], f32, tag="bcB")
                nc.gpsimd.partition_broadcast(bcB[:, :], dB[:], channels=64)
                nc.vector.tensor_mul(outN[mp][J][0:64, :], outN[mp][J][0:64, :], bc[:, :])
                # head B sits on partitions 64-127: broadcast lands at base
                # 0 (HW quirk), so DMA-shift the bcast row block up.
                bcB64 = bcast.tile([P, 512], f32, tag="bcB64")
                nc.sync.dma_start(bcB64[64:128, :], bcB[:, :])
                nc.vector.tensor_mul(outN[mp][J][64:128, :], outN[mp][J][64:128, :], bcB64[64:128, :])

            # initial qk projection for column block 0 (needed by step 0)
            for m in range(8):
                qkproj_group(m, 0)

            ou_tiles = {}
            for s in range(4):  # step = query block J = qk column block n
                fillers = []
                if s < 3:
                    fillers += [(qkproj_group, (m, s + 1)) for m in range(8)]
                if s > 0:
                    fillers += [(outproj_group, (s - 1, g)) for g in range(8)]
                nj = 4 * s + 4
                # spread fillers across the step's 4*nj attention iterations
                total_iters = 4 * nj
                stride = max(1, total_iters // max(1, len(fillers)))
                it = 0
                Js = slice(s * 512, (s + 1) * 512)
                for mp in range(4):
                    qs, ks = qkT[mp], qkT[4 + mp]
                    ouA = oups.tile([65, 512], f32, tag="ouA")
                    ouB = oups.tile([65, 512], f32, tag="ouB")
                    ou_tiles[mp] = (ouA, ouB)
                    pending = None
                    for j in range(nj):
                        sT = stps.tile([P, 1024], f32, tag="sT")
                        js = slice(j * P, (j + 1) * P)
                        nc.tensor.matmul(
                            sT[:, 0:512],
                            ks[0:64, js], qs[0:64, Js],
                            start=True, stop=True, tile_position=(0, 0),
                        )
                        nc.tensor.matmul(
                            sT[:, 512:1024],
                            ks[64:128, js], qs[64:128, Js],
                            start=True, stop=True, tile_position=(64, 0),
                        )
                        at = atp.tile([P, 1024], bf16, tag="at")
                        i = j - 4 * s
                        c0 = 128 * i if i > 0 else 0
                        if i > 0:
                            src_v = sT[:].rearrange("p (h c) -> p h c", h=2)
                            dst_v = at[:].rearrange("p (h c) -> p h c", h=2)
                            nc.scalar.activation(
                                dst_v[:, :, c0:512], src_v[:, :, c0:512],
                                EXP, bias=0.0, scale=0.125,
                            )
                        else:
                            nc.scalar.activation(at[:], sT[:], EXP, bias=0.0, scale=0.125)
                        if i >= 0:
                            # diagonal block: one strided multiply applies the
                            # triangle to both heads' [c0,c0+128)
                            atv = at[:].rearrange("p (h c) -> p h c", h=2)
                            mkv = mask_tri2[:].rearrange("p (h c) -> p h c", h=2)
                            nc.vector.tensor_mul(
                                atv[:, :, c0 : c0 + 128],
                                atv[:, :, c0 : c0 + 128],
                                mkv[:, :, :],
                            )
                        if mp == 0 and s == 0 and j == 0:
                            _dbg(nc, "at000", at[:])
                        # software-pipelined AV: one iteration behind scores
                        if pending is not None:
                            _emit_av(nc, mp, s, nj, pending, ou_tiles[mp], vv, atv_hist)
                        atv_hist[j] = at
                        pending = j
                        it += 1
                        if fillers and it % stride == 0:
                            fn, args = fillers.pop(0)
                            fn(*args)
                    _emit_av(nc, mp, s, nj, pending, ou_tiles[mp], vv, atv_hist)
                    norm_chain(mp, s)
                for fn, args in fillers:
                    fn(*args)
                if s > 0:
                    rs_chunk(s - 1)
            # final projection block + last chunk
            for g in range(8):
                outproj_group(3, g)
            rs_chunk(3)
            _dbg(nc, "qkT0", qkT[0][:])
            _dbg(nc, "rs_in", rs_in[:])


atv_hist = {}


def _emit_av(nc, mp, s, nj, j, ou, vv, hist):
    ouA, ouB = ou
    at = hist[j]
    i = j - 4 * s
    c0 = 128 * i if i > 0 else 0
    nc.tensor.matmul(
        ouA[:, c0:512], vv[j][:, 130 * mp : 130 * mp + 65],
        at[:, c0:512],
        start=(j == 0), stop=(j == nj - 1),
        skip_group_check=(i > 0 or j == nj - 1),
    )
    nc.tensor.matmul(
        ouB[:, c0:512], vv[j][:, 130 * mp + 65 : 130 * mp + 130],
        at[:, 512 + c0 : 1024],
        start=(j == 0), stop=(j == nj - 1),
        skip_group_check=(i > 0 or j == nj - 1),
    )


def _build():
    if "nc" in _CACHE:
        return _CACHE["nc"]
    nc = bacc.Bacc("TRN2", target_bir_lowering=False, debug=False, num_devices=NCORES)
    x_d = nc.dram_tensor("x", [T, D], bf16, kind="ExternalInput")
    wqk_d = nc.dram_tensor("w_qk", [D, 1024], bf16, kind="ExternalInput")
    wv_d = nc.dram_tensor("w_v", [D, 512], bf16, kind="ExternalInput")
    bqk_d = nc.dram_tensor("b_qk", [8, P, 1], f32, kind="ExternalInput")
    wproj_d = nc.dram_tensor("w_proj", [512, D], bf16, kind="ExternalInput")
    beta_d = nc.dram_tensor("beta", [1, D], bf16, kind="ExternalInput")
    out_d = nc.dram_tensor("out", [T // 2, D], bf16, kind="ExternalOutput")
    with tile.TileContext(nc) as tc:
        _emit(nc, tc, x_d, wqk_d, wv_d, bqk_d, wproj_d, beta_d, out_d)
    nc.compile()
    _CACHE["nc"] = nc
    return nc


def make_in_maps(x, w_qkv, b_qkv, w_proj, b_proj):
    x = np.asarray(x, np.float32)
    w_qkv = np.asarray(w_qkv, np.float32)
    b_qkv = np.asarray(b_qkv, np.float32)
    w_proj = np.asarray(w_proj, np.float32)
    b_proj = np.asarray(b_proj, np.float32)
    in_maps = []
    for c in range(NCORES):
        b, g = c // 2, c % 2
        qcols = slice(g * 512, (g + 1) * 512)
        kcols = slice(D + g * 512, D + (g + 1) * 512)
        vcols = slice(2 * D + g * 512, 2 * D + (g + 1) * 512)
        w_qk = np.concatenate([w_qkv[:, qcols], w_qkv[:, kcols]], axis=1)
        b_qk = np.concatenate([b_qkv[qcols], b_qkv[kcols]])
        wp = np.ascontiguousarray(w_proj[g * 512 : (g + 1) * 512, :])
        beta = wp.T @ b_qkv[vcols]
        if g == 0:
            beta = beta + b_proj
        in_maps.append({
            "x": np.ascontiguousarray(x[b]).astype(ml_dtypes.bfloat16),
            "w_qk": np.ascontiguousarray(w_qk).astype(ml_dtypes.bfloat16),
            "w_v": np.ascontiguousarray(w_qkv[:, vcols]).astype(ml_dtypes.bfloat16),
            "b_qk": b_qk.reshape(8, P, 1),
            "w_proj": wp.astype(ml_dtypes.bfloat16),
            "beta": beta.reshape(1, D).astype(ml_dtypes.bfloat16),
        })
    return in_maps


def kernel(x, w_qkv, b_qkv, w_proj, b_proj, trace=False, **run_kwargs):
    global LAST_RESULTS
    nc = _build()
    in_maps = make_in_maps(x, w_qkv, b_qkv, w_proj, b_proj)
    res = run_bass_kernel_spmd(
        nc, in_maps, core_ids=list(range(NCORES)), trace=trace, **run_kwargs
    )
    LAST_RESULTS = res
    out = np.empty((B, T, D), np.float32)
    for b in range(B):
        ev = np.asarray(res.results[2 * b]["out"], np.float32)
        od = np.asarray(res.results[2 * b + 1]["out"], np.float32)
        for J in range(4):
            out[b, 512 * J : 512 * J + 256] = ev[256 * J : 256 * (J + 1)]
            out[b, 512 * J + 256 : 512 * (J + 1)] = od[256 * J : 256 * (J + 1)]
    return out


# revision 11
# speedup vs baseline: 1.5949x; 1.1360x over previous
"""Causal self-attention Bass kernel for 8 trn2 NeuronCores.

Problem: B=4, T=2048, D=1024, H=16 causal self-attention (qkv proj + attn + out proj).

Sharding: core c = 2*b + g handles batch b (=c//2) and head-group g (=c%2, 8 heads).
Per core:
  - qkv projection column-shard: q,k,v columns for its 8 heads only.
  - flash-style attention in transposed-score layout sT[tk, tq]; softmax denominator
    via an extra ones-column in the AV matmul (row 64 of the [65, 512] psum output).
  - output projection row-shard (w_proj rows for its head dims) -> partial [T, D].
  - pairwise ReduceScatter {2b, 2b+1} sums the two head-group partials; chunked
    over 4 query blocks so the collective overlaps the attention/projection
    stream.  Even core ends with global rows {512J..512J+256}, odd core with
    {512J+256..512J+512}; host reassembles.

Pipeline: phase 2 runs query-block-outer (J = 0..3).  Step J computes attention
for all 4 head-pairs at query block J, while interleaving (as PE filler work
between ACT-gated attention iterations) the qk-projection for column block J+1
and the output projection + ReduceScatter chunk for block J-1.  The AV matmul is
software-pipelined one iteration behind the score matmul so the PE never waits
on the exp/mask chain.

Precision: bf16 operands everywhere on the matmul paths (1 cyc/row streaming,
FWL weight loads); psum accumulation stays f32.  The ReduceScatter runs in bf16;
host converts the output to f32.  b_v is folded into beta = b_proj(once per
pair) + w_proj_shard.T @ b_v_shard since softmax rows sum to 1.

Diagonal blocks: partial exp (cols >= 128*i only) and ONE strided triangle-mask
multiply covering both heads; AV matmuls read only the valid column range, so no
zero-fill of the masked region is needed.
"""

from contextlib import ExitStack

import ml_dtypes
import numpy as np

import concourse.bass as bass
import concourse.mybir as mybir
import concourse.tile as tile
from concourse import bacc
from concourse.bass_utils import run_bass_kernel_spmd

B, T, D, H = 4, 2048, 1024, 16
HD = D // H  # 64
NCORES = 8
P = 128
f32 = mybir.dt.float32
f32r = mybir.dt.float32r
bf16 = mybir.dt.bfloat16
EXP = mybir.ActivationFunctionType.Exp

_CACHE = {}
LAST_RESULTS = None
_DEBUG_SINK = None


def _dbg(nc, name, ap):
    if _DEBUG_SINK is not None and name in _DEBUG_SINK:
        nc.sync.dma_start(_DEBUG_SINK[name].ap(), ap)


def _emit(nc, tc, x_d, wqk_d, wv_d, bqk_d, wproj_d, beta_d, out_d):
    with ExitStack() as ctx:
        # ---------------- constants / persistent tiles ----------------
        const = ctx.enter_context(tc.tile_pool(name="const", bufs=1))
        bootc = ctx.enter_context(tc.tile_pool(name="boot", bufs=1))
        ident_f = bootc.tile([P, P], bf16, tag="ident_f")
        nc.gpsimd.memset(ident_f[:], 0.0)
        nc.gpsimd.affine_select(
            out=ident_f[:], in_=ident_f[:],
            compare_op=mybir.AluOpType.not_equal, fill=1.0,
            base=0, pattern=[[-1, P]], channel_multiplier=1,
        )
        # triangle mask [128,256] = two copies of (keep iff f >= p)
        mask_tri2 = const.tile([P, 2 * P], bf16, tag="mask_tri2")
        nc.gpsimd.memset(mask_tri2[:], 1.0)
        for h0 in (0, P):
            nc.gpsimd.affine_select(
                out=mask_tri2[:, h0 : h0 + P], in_=mask_tri2[:, h0 : h0 + P],
                compare_op=mybir.AluOpType.is_ge, fill=0.0,
                base=0, pattern=[[1, P]], channel_multiplier=-1,
            )
        bq8 = const.tile([P, 8], f32, tag="bq8")
        beta_b = const.tile([P, D], bf16, tag="beta_b")

        def _load_small_consts():
            nc.sync.dma_start(bq8[:], bqk_d.ap().rearrange("m p one -> p (m one)"))
            nc.sync.dma_start(beta_b[0:1, :], beta_d.ap())
            nc.gpsimd.partition_broadcast(beta_b[:], beta_b[0:1, :], channels=P)
        # weight pools reserved here; DMAs are emitted after phase 1 starts
        # so the x loads win the DMA queue.
        wpp = ctx.enter_context(tc.tile_pool(name="wpp", bufs=1))
        wproj_t = [wpp.tile([P, D], bf16, tag=f"wp{hp}", name=f"wp{hp}") for hp in range(4)]
        wqkp = ctx.enter_context(tc.tile_pool(name="wqk", bufs=1))
        # one [P, 1024] tile per k-chunk holding all 8 m column blocks
        wq8 = [wqkp.tile([P, 1024], bf16, tag=f"wq8k{k}", name=f"wq8k{k}") for k in range(8)]
        _dbg(nc, "beta_b", beta_b[:])

        # persistent activations
        xt_pool = ctx.enter_context(tc.tile_pool(name="xt", bufs=1))
        xT = [xt_pool.tile([P, T], bf16, tag=f"xT{k}", name=f"xT{k}") for k in range(8)]
        vv_pool = ctx.enter_context(tc.tile_pool(name="vv", bufs=1))
        vv = [vv_pool.tile([P, 520], bf16, tag=f"vv{i}", name=f"vv{i}") for i in range(16)]
        on_pool = ctx.enter_context(tc.tile_pool(name="outn", bufs=1))
        outN = [[on_pool.tile([P, 512], bf16, tag=f"outN{mp}J{J}", name=f"outN{mp}J{J}")
                 for J in range(4)] for mp in range(4)]
        ones8 = const.tile([P, 8], bf16, tag="ones8")
        nc.vector.memset(ones8[:], 1.0)
        ones_src = ones8[:].rearrange("p (mp h one) -> p mp h one", mp=4, h=2)
        for i in range(16):
            dst = vv[i][:].rearrange("p (mp h d) -> p mp h d", mp=4, h=2)
            nc.vector.tensor_copy(dst[:, :, :, 64:65], ones_src[:, :, :, :])

        dram = ctx.enter_context(tc.tile_pool(name="dram", bufs=1, space="DRAM"))
        rs_in = dram.tile([T, D], bf16)
        rs_out = dram.tile([T // 2, D], bf16)
        dram2 = ctx.enter_context(tc.tile_pool(name="dram2", bufs=2, space="DRAM"))

        # ---------------- phase 1: load x, transpose, compute v ----------------
        with ExitStack() as p1:
            xload = p1.enter_context(tc.tile_pool(name="xload", bufs=3))
            wvp = p1.enter_context(tc.tile_pool(name="wv", bufs=1))
            tpps = p1.enter_context(tc.tile_pool(name="tpps", bufs=2, space="PSUM"))
            vps = p1.enter_context(tc.tile_pool(name="vps", bufs=2, space="PSUM"))
            # [P, 1024] tiles, each holding two k-chunks side by side
            wv_t = [wvp.tile([P, 1024], bf16, tag=f"wvt{k2}", name=f"wvt{k2}") for k2 in range(4)]
            for qq in range(4):  # t-quarters
                # one [P, 2048] load covers two t-tiles (rows r0..r0+256)
                xi2 = []
                for ii in range(2):
                    xt_ = xload.tile([P, 2 * D], bf16, tag="x")
                    r0 = (qq * 4 + 2 * ii) * P
                    nc.sync.dma_start(
                        xt_[:].rearrange("p (b c) -> p b c", b=2),
                        x_d.ap()[r0 : r0 + 2 * P, :].rearrange("(b p) c -> p b c", p=P),
                    )
                    xi2.append(xt_)
                xi = [
                    xi2[ii // 2][:, (ii % 2) * D : (ii % 2 + 1) * D]
                    for ii in range(4)
                ]
                if qq == 0:
                    # weight loads queue after the first x tiles
                    for k2 in range(4):
                        nc.sync.dma_start(
                            wv_t[k2][:].rearrange("p (b c) -> p b c", b=2),
                            wv_d.ap()[k2 * 2 * P : (k2 + 1) * 2 * P, :].rearrange(
                                "(b p) c -> p b c", p=P
                            ),
                        )
                    for hp in range(4):
                        nc.sync.dma_start(
                            wproj_t[hp][:], wproj_d.ap()[hp * P : (hp + 1) * P, :]
                        )
                    _load_small_consts()
                if qq == 1:
                    for k in range(8):
                        nc.sync.dma_start(wq8[k][:], wqk_d.ap()[k * P : (k + 1) * P, :])
                for k in range(8):
                    tp = tpps.tile([P, 512], bf16, tag="tp")
                    for ii in range(4):
                        nc.tensor.transpose(
                            tp[:, ii * P : (ii + 1) * P],
                            xi[ii][:, k * P : (k + 1) * P],
                            ident_f[:],
                        )
                    nc.scalar.copy(xT[k][:, qq * 512 : (qq + 1) * 512], tp[:])
                # v for this quarter's 4 t-tiles
                for il in range(4):
                    i = qq * 4 + il
                    ps = vps.tile([P, 512], f32, tag="vp")
                    for k in range(8):
                        nc.tensor.matmul(
                            ps[:],
                            xT[k][:, i * P : (i + 1) * P],
                            wv_t[k // 2][:, (k % 2) * 512 : (k % 2 + 1) * 512],
                            start=(k == 0), stop=(k == 7),
                        )
                    # strided evict: psum [p, (mp h d)] d=64 -> vv [p, (mp h d65)]
                    src = ps[:].rearrange("p (mp h d) -> p mp h d", mp=4, h=2)
                    dst = vv[i][:].rearrange("p (mp h d) -> p mp h d", mp=4, h=2)
                    nc.vector.tensor_copy(dst[:, :, :, 0:64], src[:, :, :, :])
            _dbg(nc, "xT0", xT[0][:])
            _dbg(nc, "vv0", vv[0][:])

        # ---------------- phase 2: J-outer attention pipeline ----------------
        with ExitStack() as p2:
            qkt_pool = p2.enter_context(tc.tile_pool(name="qkt", bufs=1))
            qkT = [qkt_pool.tile([P, T], bf16, tag=f"qkT{m}", name=f"qkT{m}") for m in range(8)]
            atp = p2.enter_context(tc.tile_pool(name="atp", bufs=3))
            recip = p2.enter_context(tc.tile_pool(name="recip", bufs=2))
            bcast = p2.enter_context(tc.tile_pool(name="bcast", bufs=2))
            tmpb = p2.enter_context(tc.tile_pool(name="tmpb", bufs=2))
            qkps = p2.enter_context(tc.tile_pool(name="qkps", bufs=2, space="PSUM"))
            stps = p2.enter_context(tc.tile_pool(name="stps", bufs=2, space="PSUM"))
            oups = p2.enter_context(tc.tile_pool(name="oups", bufs=1, space="PSUM"))

            def qkproj_group(m, n):
                ps = qkps.tile([P, 512], f32, tag="qkp")
                for k in range(8):
                    nc.tensor.matmul(
                        ps[:], wq8[k][:, m * P : (m + 1) * P],
                        xT[k][:, n * 512 : (n + 1) * 512],
                        start=(k == 0), stop=(k == 7),
                    )
                nc.vector.tensor_scalar_add(
                    qkT[m][:, n * 512 : (n + 1) * 512], ps[:], bq8[:, m : m + 1]
                )

            def outproj_group(J, g):
                i = 4 * J + g // 2
                n = g % 2
                ps = qkps.tile([P, 512], f32, tag="qkp", name="fp")
                for hp in range(4):
                    nc.tensor.matmul(
                        ps[:],
                        outN[hp][J][:, (i % 4) * P : (i % 4 + 1) * P],
                        wproj_t[hp][:, n * 512 : (n + 1) * 512],
                        start=(hp == 0), stop=(hp == 3),
                    )
                fin = atp.tile([P, 512], bf16, tag="fin")
                nc.vector.tensor_add(fin[:], ps[:], beta_b[:, n * 512 : (n + 1) * 512])
                nc.sync.dma_start(
                    rs_in[i * P : (i + 1) * P, n * 512 : (n + 1) * 512], fin[:]
                )

            def rs_chunk(J):
                if globals().get("_NO_COLLECTIVE"):
                    nc.sync.dma_start(
                        out_d.ap()[256 * J : 256 * J + 256, :],
                        rs_in[512 * J : 512 * J + 256, :],
                    )
                    return
                nc.gpsimd.collective_compute(
                    "ReduceScatter", mybir.AluOpType.add,
                    replica_groups=[[0, 1], [2, 3], [4, 5], [6, 7]],
                    ins=[rs_in[512 * J : 512 * (J + 1), :].opt()],
                    outs=[rs_out[256 * J : 256 * (J + 1), :].opt()],
                )

            def out_dma(J):
                # deferred to kernel end: these wait on RS completion, so they
                # must not sit ahead of compute-feeding DMAs in the sync queue
                if globals().get("_NO_COLLECTIVE"):
                    return
                nc.sync.dma_start(
                    out_d.ap()[256 * J : 256 * (J + 1), :],
                    rs_out[256 * J : 256 * (J + 1), :],
                )

            def norm_chain(mp, J):
                # Lazy normalization: raw-evict values + denominators so the
                # psum slots free fast, then reciprocals via a DRAM repack
                # ([1,1024] -> [128,8]) so the DVE iterative divide runs on all
                # lanes.  outN is only read by the (one-step-later) projection,
                # so this chain is off the attention critical path.
                ouA, ouB = ou_tiles[mp]
                dd = recip.tile([1, 1024], f32, tag="dd")
                tb = tmpb.tile([64, 512], bf16, tag="tb")
                nc.vector.tensor_copy(dd[:, 0:512], ouA[64:65, :])
                nc.vector.tensor_copy(outN[mp][J][0:64, :], ouA[0:64, :])
                nc.vector.tensor_copy(dd[:, 512:1024], ouB[64:65, :])
                nc.vector.tensor_copy(tb[:], ouB[0:64, :])
                nc.sync.dma_start(outN[mp][J][64:128, :], tb[:])
                dramD = dram2.tile([1, 1024], f32, tag="dramD", name="dramD")
                nc.sync.dma_start(dramD[:], dd[:])
                dPack = recip.tile([P, 8], f32, tag="dPack")
                nc.sync.dma_start(dPack[:], dramD[:].rearrange("a (p c) -> (a p) c", p=P))
                nc.vector.reciprocal(dPack[:], dPack[:])
                nc.sync.dma_start(dramD[:].rearrange("a (p c) -> (a p) c", p=P), dPack[:])
                nc.sync.dma_start(dd[:], dramD[:])
                # full-width broadcasts: head A uses rows 0:64 of bc, head B
                # rows 64:128 of bcB (partition_broadcast fills all channels)
                bc = bcast.tile([64, 512], f32, tag="bc")
                nc.gpsimd.partition_broadcast(bc[:, :], dd[:, 0:512], channels=64)
                bcB = bcast.tile([P, 512], f32, tag="bcB")
                nc.gpsimd.partition_broadcast(bcB[:, :], dd[:, 512:1024], channels=P)
                nc.vector.tensor_mul(outN[mp][J][0:64, :], outN[mp][J][0:64, :], bc[:, :])
                nc.vector.tensor_mul(outN[mp][J][64:128, :], outN[mp][J][64:128, :], bcB[64:128, :])

            # initial qk projection for column block 0 (needed by step 0)
            for m in range(8):
                qkproj_group(m, 0)

            ou_tiles = {}
            for s in range(4):  # step = query block J = qk column block n
                fillers = []
                if s < 3:
                    fillers += [(qkproj_group, (m, s + 1)) for m in range(8)]
                if s > 0:
                    fillers += [(outproj_group, (s - 1, g)) for g in range(8)]
                nj = 4 * s + 4
                # spread fillers across the step's 4*nj attention iterations
                total_iters = 4 * nj
                stride = max(1, total_iters // max(1, len(fillers)))
                it = 0
                Js = slice(s * 512, (s + 1) * 512)
                for mp in range(4):
                    qs, ks = qkT[mp], qkT[4 + mp]
                    ouA = oups.tile([65, 512], f32, tag="ouA")
                    ouB = oups.tile([65, 512], f32, tag="ouB")
                    ou_tiles[mp] = (ouA, ouB)
                    pending = None
                    for j in range(nj):
                        sT = stps.tile([P, 1024], f32, tag="sT")
                        js = slice(j * P, (j + 1) * P)
                        nc.tensor.matmul(
                            sT[:, 0:512],
                            ks[0:64, js], qs[0:64, Js],
                            start=True, stop=True, tile_position=(0, 0),
                        )
                        nc.tensor.matmul(
                            sT[:, 512:1024],
                            ks[64:128, js], qs[64:128, Js],
                            start=True, stop=True, tile_position=(64, 0),
                        )
                        at = atp.tile([P, 1024], bf16, tag="at")
                        i = j - 4 * s
                        c0 = 128 * i if i > 0 else 0
                        if i > 0:
                            src_v = sT[:].rearrange("p (h c) -> p h c", h=2)
                            dst_v = at[:].rearrange("p (h c) -> p h c", h=2)
                            nc.scalar.activation(
                                dst_v[:, :, c0:512], src_v[:, :, c0:512],
                                EXP, bias=0.0, scale=0.125,
                            )
                        else:
                            nc.scalar.activation(at[:], sT[:], EXP, bias=0.0, scale=0.125)
                        if i >= 0:
                            # diagonal block: one strided multiply applies the
                            # triangle to both heads' [c0,c0+128)
                            atv = at[:].rearrange("p (h c) -> p h c", h=2)
                            mkv = mask_tri2[:].rearrange("p (h c) -> p h c", h=2)
                            nc.vector.tensor_mul(
                                atv[:, :, c0 : c0 + 128],
                                atv[:, :, c0 : c0 + 128],
                                mkv[:, :, :],
                            )
                        if mp == 0 and s == 0 and j == 0:
                            _dbg(nc, "at000", at[:])
                        # software-pipelined AV: one iteration behind scores
                        if pending is not None:
                            _emit_av(nc, mp, s, nj, pending, ou_tiles[mp], vv, atv_hist)
                        atv_hist[j] = at
                        pending = j
                        it += 1
                        if fillers and it % stride == 0:
                            fn, args = fillers.pop(0)
                            fn(*args)
                    _emit_av(nc, mp, s, nj, pending, ou_tiles[mp], vv, atv_hist)
                    norm_chain(mp, s)
                for fn, args in fillers:
                    fn(*args)
                if s > 0:
                    rs_chunk(s - 1)
            # final projection block + last chunk
            for g in range(8):
                outproj_group(3, g)
            rs_chunk(3)
            for J in range(4):
                out_dma(J)
            _dbg(nc, "qkT0", qkT[0][:])
            _dbg(nc, "rs_in", rs_in[:])


atv_hist = {}


def _emit_av(nc, mp, s, nj, j, ou, vv, hist):
    ouA, ouB = ou
    at = hist[j]
    i = j - 4 * s
    c0 = 128 * i if i > 0 else 0
    nc.tensor.matmul(
        ouA[:, c0:512], vv[j][:, 130 * mp : 130 * mp + 65],
        at[:, c0:512],
        start=(j == 0), stop=(j == nj - 1),
        skip_group_check=(i > 0 or j == nj - 1),
    )
    nc.tensor.matmul(
        ouB[:, c0:512], vv[j][:, 130 * mp + 65 : 130 * mp + 130],
        at[:, 512 + c0 : 1024],
        start=(j == 0), stop=(j == nj - 1),
        skip_group_check=(i > 0 or j == nj - 1),
    )


def _build():
    if "nc" in _CACHE:
        return _CACHE["nc"]
    nc = bacc.Bacc("TRN2", target_bir_lowering=False, debug=False, num_devices=NCORES)
    x_d = nc.dram_tensor("x", [T, D], bf16, kind="ExternalInput")
    wqk_d = nc.dram_tensor("w_qk", [D, 1024], bf16, kind="ExternalInput")
    wv_d = nc.dram_tensor("w_v", [D, 512], bf16, kind="ExternalInput")
    bqk_d = nc.dram_tensor("b_qk", [8, P, 1], f32, kind="ExternalInput")
    wproj_d = nc.dram_tensor("w_proj", [512, D], bf16, kind="ExternalInput")
    beta_d = nc.dram_tensor("beta", [1, D], bf16, kind="ExternalInput")
    out_d = nc.dram_tensor("out", [T // 2, D], bf16, kind="ExternalOutput")
    with tile.TileContext(nc) as tc:
        _emit(nc, tc, x_d, wqk_d, wv_d, bqk_d, wproj_d, beta_d, out_d)
    nc.compile()
    _CACHE["nc"] = nc
    return nc


def make_in_maps(x, w_qkv, b_qkv, w_proj, b_proj):
    x = np.asarray(x, np.float32)
    w_qkv = np.asarray(w_qkv, np.float32)
    b_qkv = np.asarray(b_qkv, np.float32)
    w_proj = np.asarray(w_proj, np.float32)
    b_proj = np.asarray(b_proj, np.float32)
    in_maps = []
    for c in range(NCORES):
        b, g = c // 2, c % 2
        qcols = slice(g * 512, (g + 1) * 512)
        kcols = slice(D + g * 512, D + (g + 1) * 512)
        vcols = slice(2 * D + g * 512, 2 * D + (g + 1) * 512)
        w_qk = np.concatenate([w_qkv[:, qcols], w_qkv[:, kcols]], axis=1)
        b_qk = np.concatenate([b_qkv[qcols], b_qkv[kcols]])
        wp = np.ascontiguousarray(w_proj[g * 512 : (g + 1) * 512, :])
        beta = wp.T @ b_qkv[vcols]
        if g == 0:
            beta = beta + b_proj
        in_maps.append({
            "x": np.ascontiguousarray(x[b]).astype(ml_dtypes.bfloat16),
            "w_qk": np.ascontiguousarray(w_qk).astype(ml_dtypes.bfloat16),
            "w_v": np.ascontiguousarray(w_qkv[:, vcols]).astype(ml_dtypes.bfloat16),
            "b_qk": b_qk.reshape(8, P, 1),
            "w_proj": wp.astype(ml_dtypes.bfloat16),
            "beta": beta.reshape(1, D).astype(ml_dtypes.bfloat16),
        })
    return in_maps


def kernel(x, w_qkv, b_qkv, w_proj, b_proj, trace=False, **run_kwargs):
    global LAST_RESULTS
    nc = _build()
    in_maps = make_in_maps(x, w_qkv, b_qkv, w_proj, b_proj)
    res = run_bass_kernel_spmd(
        nc, in_maps, core_ids=list(range(NCORES)), trace=trace, **run_kwargs
    )
    LAST_RESULTS = res
    out = np.empty((B, T, D), np.float32)
    for b in range(B):
        ev = np.asarray(res.results[2 * b]["out"], np.float32)
        od = np.asarray(res.results[2 * b + 1]["out"], np.float32)
        for J in range(4):
            out[b, 512 * J : 512 * J + 256] = ev[256 * J : 256 * (J + 1)]
            out[b, 512 * J + 256 : 512 * (J + 1)] = od[256 * J : 256 * (J + 1)]
    return out
